# revision 84
# baseline (speedup 1.0000x reference)
"""BitNet Llama MLP on 8 trn2 NeuronCores (Bass/Tile) — fp8 DoubleRow, local-first.

y = bitlinear(silu(bitlinear(x, w_gate)) * bitlinear(x, w_up), w_down)

Scheme (see kernel_baseline.py for the ancestor):
  * All fake-quantized values are exact in fp8 planes: ternary weights in
    e4m3 directly; int8 activations split exactly as q = hi + lo with
    hi = round16(q), |lo| <= 8.  All matmuls run fp8 DoubleRow (2x bf16 MAC
    rate) with fp32 PSUM accumulation -> numerics match the int8 scheme.
  * Token-data-parallel: each core owns Tc=1024 tokens end to end; the
    quantized fp8 weights are replicated via AllGather (the collectives'
    high-bandwidth operating point; no per-token collectives).
  * Local-first phase B: each core starts gate/up matmuls on its OWN 11
    quantized I-blocks straight from its local staging buffer (no AllGather
    wait), filling the PE idle window while the AG chain runs.  Remote
    blocks are processed in AG-landing order via a relative-core rotation
    c = (pid + r) & 7 with register-offset DMA addressing, so the SPMD
    program needs no per-core specialization.
  * One merged AllReduce carries all three |w| sums (g, u, d): the wd sum
    joins the prepass reads, eliminating a second AR whose result readback
    otherwise head-of-line blocks whichever queue hosts it.
  * AG chunking (3,4,4) blocks/chunk keeps transfers near the collective
    model's high-bandwidth point; the chain AR -> AG0..AG2 -> AGwd runs
    gapless, fully overlapped with quant + matmuls.
  * Queue discipline: bulk streaming loads on SP, producer-gated stores on
    Act, tiny AR/scale DMAs on Pool next to their collectives - DMA waits
    block the issuing sequencer, so a blocked readback must never sit ahead
    of critical loads.  Partition reductions for the AR go through a DMA
    round-trip (column -> DRAM -> row) rather than a PE transpose, keeping
    the PE queue free of long waits.
  * PSUM: the gate/up matmul pool spans phases A+B (4 banks reserved up
    front); quant/x transposes pack 4-8 per bank so both fit alongside.
  * x planes are split per token-half so half-0 matmuls depend only on
    token blocks 0-3's quant; x and h quant skip the int8 clamp (values
    are <=127 by construction since scale = 127/absmax).
  * h is staged to DRAM in fp32 per token-half (separate tensors per half
    avoid WAR serialization between half-1 stores and half-0 quant reads).
    hq(half0) overlaps the last chunk's half-1 matmuls; hq(half1) overlaps
    the down-proj of token-half 0.  The down-proj runs plane-outer (exact
    integer PSUM accumulation makes reordering bit-identical), so each
    token-group's matmuls start after only the hi-plane load and the
    lo-plane load hides under them.
"""

import sys

sys.path.insert(0, "/opt/trn_rl_repo")

import numpy as np

import concourse.bass as bass
import concourse.bacc as bacc
import concourse.mybir as mybir
import concourse.tile as tile
from concourse.bass_utils import run_bass_kernel_spmd
from concourse.masks import make_identity

F32 = mybir.dt.float32
BF16 = mybir.dt.bfloat16
FP8 = mybir.dt.float8e4
MAGIC = 12582912.0      # 1.5*2^23: fp32 add/sub rounds to nearest int
MAGIC16 = MAGIC * 16.0  # 1.5*2^27: rounds to nearest multiple of 16
EPS = 1e-5
N_CORES = 8
DR = mybir.MatmulPerfMode.DoubleRow

FULL_CFG = dict(H=4096, Tc=1024, Ish=1408, count=11008 * 4096, chunks=(3, 4, 4))


def build_program(H, Tc, Ish, count, chunks):
    n_hb = H // 128            # H 128-blocks (contraction for gate/up)
    n_ishl = Ish // 128        # local I blocks per shard
    n_ib = N_CORES * n_ishl    # global I blocks
    half_T = Tc // 2
    n_tt = Tc // 128
    assert n_hb % 2 == 0 and n_ib % 2 == 0 and sum(chunks) == n_ishl
    rg = [list(range(N_CORES))]
    AX = mybir.AxisListType.X
    OP = mybir.AluOpType
    ACT = mybir.ActivationFunctionType
    ENG = mybir.EngineType

    nc = bacc.Bacc("TRN2", target_bir_lowering=False, debug=False,
                   num_devices=N_CORES)

    x_s = nc.dram_tensor("x_s", [Tc, H], F32, kind="ExternalInput")
    wg_s = nc.dram_tensor("wg_s", [Ish, H], F32, kind="ExternalInput")
    wu_s = nc.dram_tensor("wu_s", [Ish, H], F32, kind="ExternalInput")
    wd_s = nc.dram_tensor("wd_s", [H, Ish], F32, kind="ExternalInput")
    y = nc.dram_tensor("y", [Tc, H], F32, kind="ExternalOutput")

    with tile.TileContext(nc) as tc:
        with (
            tc.tile_pool(name="const", bufs=1) as cpool,
            tc.tile_pool(name="dram", bufs=1, space="DRAM") as dram,
        ):
            # ---------------- DRAM intermediates ----------------
            wgu8_c = [dram.tile([128, nb, 2, n_hb, 128], FP8, tag=f"wgu8_{k}",
                                name=f"wgu8_{k}")
                      for k, nb in enumerate(chunks)]
            wgu8_all = [dram.tile([N_CORES, 128, nb, 2, n_hb, 128], FP8,
                                  tag=f"wgu8a_{k}", name=f"wgu8a_{k}",
                                  addr_space="Shared")
                        for k, nb in enumerate(chunks)]
            wd8_s = dram.tile([128, n_ishl, H], FP8, tag="wd8_s")
            wd8_all = dram.tile([N_CORES, 128, n_ishl, H], FP8, tag="wd8a",
                                name="wd8a", addr_space="Shared")
            ar_in = dram.tile([1, 3], F32, tag="ar_in")
            ar_out = dram.tile([1, 3], F32, tag="ar_out",
                               name="ar_out", addr_space="Shared")
            gusum_dr = dram.tile([128, 3], F32, tag="gusum_dr")
            deq_row = dram.tile([1, Tc], F32, tag="deq_row")
            habs_row = dram.tile([1, Tc], F32, tag="habs_row")
            h_st = [dram.tile([n_ib, 128, half_T], F32, tag=f"h_st{hf}",
                              name=f"h_st{hf}") for hf in range(2)]
            hq_st = [dram.tile([n_ib, 128, 2, half_T], FP8, tag=f"hq_st{hf}",
                               name=f"hq_st{hf}") for hf in range(2)]
            hq_v = [hq_st[hf].rearrange("b p two t -> two p b t")
                    for hf in range(2)]

            idb = cpool.tile([128, 128], BF16, tag="idb")
            make_identity(nc, idb[:])
            idf = cpool.tile([128, 128], F32, tag="idf")
            make_identity(nc, idf[:])
            # per-tensor scale tiles: separate wd from gate/up so the late
            # wd AllReduce broadcast creates no false tile-level deps on
            # gate/up consumers
            scGU = cpool.tile([128, 2], F32, tag="scGU")  # sw_g, sw_u
            rcGU = cpool.tile([128, 2], F32, tag="rcGU")
            scD = cpool.tile([128, 1], F32, tag="scD")    # sw_d
            rcD = cpool.tile([128, 1], F32, tag="rcD")
            dv8 = cpool.tile([128, n_tt], F32, tag="dv8")

            # partition id on SP (loads/stores) + Act (h/hq stores)
            pid = nc.partition_id(engines=[ENG.SP, ENG.Activation])

            # chunk boundaries: chunk k covers local blocks [lo_k, hi_k)
            los = [sum(chunks[:k]) for k in range(len(chunks))]

            # processing order: 11 local ents, then remote in chunk-major
            # rotation order.  ent = ("L", k, l) local or ("R", k, r, l).
            order = [("L", k, l) for k in range(len(chunks))
                     for l in range(los[k], los[k] + chunks[k])]
            for k in range(len(chunks)):
                for r in range(1, N_CORES):
                    for l in range(los[k], los[k] + chunks[k]):
                        order.append(("R", k, r, l))
            assert len(order) == n_ib

            def ent_slot(ent):
                # dynamic global block index for h staging
                if ent[0] == "L":
                    return pid * n_ishl + ent[2]
                _, k, r, l = ent
                return ((pid + r) & (N_CORES - 1)) * n_ishl + l

            def ent_name(ent):
                return "_".join(str(p) for p in ent)

            # PSUM->SBUF copies: only DVE and Act can read PSUM
            cp_engines = (nc.vector.tensor_copy, nc.scalar.copy)

            # quant pools span phases B and C (half-1 h quant interleaves
            # with the down-proj of half 0)
            with (
                tc.tile_pool(name="pq", bufs=4) as pq,
                tc.tile_pool(name="pq1", bufs=2) as pq1,
            ):
                qsb = [None, None]

                def emit_habs(half, maxacc, pbe, pbpt):
                    nq = half_T // 128
                    tpq = pbpt.tile([128, nq, 128], F32, tag="tpf",
                                    name=f"mtp{half}")
                    for q in range(nq):
                        tok0 = half * half_T + q * 128
                        nc.tensor.transpose(
                            tpq[:, q, :], maxacc[:, q * 128:(q + 1) * 128],
                            idf[:])
                        red = pbe.tile([128, 1], F32, tag="red",
                                       name=f"red{half}_{q}")
                        nc.vector.tensor_reduce(red[:], tpq[:, q, :],
                                                axis=AX, op=OP.max)
                        redc = pbe.tile([128, 1], F32, tag="redc",
                                        name=f"redc{half}_{q}")
                        nc.vector.tensor_scalar(redc[:], red[:], EPS, 0.0,
                                                op0=OP.max, op1=OP.add)
                        col = half * nq + q
                        nc.gpsimd.tensor_scalar(
                            dv8[:, col:col + 1], redc[:], scD[:, 0:1],
                            1.0 / 127.0, op0=OP.mult, op1=OP.mult)
                        nc.gpsimd.dma_start(
                            habs_row[0, tok0:tok0 + 128], redc[:])
                    hrow = pq1.tile([1, half_T], F32, tag="hrow",
                                    name=f"hr{half}")
                    nc.gpsimd.dma_start(
                        hrow[:],
                        habs_row[0, half * half_T:(half + 1) * half_T])
                    qsb[half] = pq1.tile([128, half_T], F32, tag="qsb",
                                         name=f"qsb{half}")
                    nc.gpsimd.partition_broadcast(qsb[half][:], hrow[:])
                    nc.vector.reciprocal(qsb[half][:], qsb[half][:])
                    nc.vector.tensor_scalar(qsb[half][:], qsb[half][:],
                                            127.0, 0.0, op0=OP.mult,
                                            op1=OP.add)

                def emit_hq_block(half, g_):
                    # absolute slot g_ (static): quantize h[g_, :, half]
                    nm = f"q{half}_{g_}"
                    hl = pq.tile([128, half_T], F32, tag="hl", name=f"hl{nm}")
                    nc.sync.dma_start(hl[:], h_st[half][g_, :, :])
                    # |h·qsb| <= 127 by construction (qsb=127/habs max),
                    # so the int8 clamp is dead code here
                    nc.vector.tensor_tensor(hl[:], hl[:], qsb[half][:],
                                            op=OP.mult)
                    nc.gpsimd.tensor_scalar(hl[:], hl[:], MAGIC, MAGIC,
                                            op0=OP.add, op1=OP.subtract)
                    hh32 = pq.tile([128, half_T], F32, tag="hh32",
                                   name=f"hh32{nm}")
                    nc.gpsimd.tensor_scalar(hh32[:], hl[:], MAGIC16,
                                            MAGIC16, op0=OP.add,
                                            op1=OP.subtract)
                    hq8 = pq.tile([128, 2, half_T], FP8, tag="hq8",
                                  name=f"hq8{nm}")
                    nc.scalar.copy(hq8[:, 0, :], hh32[:])
                    nc.vector.tensor_tensor(hq8[:, 1, :], hl[:], hh32[:],
                                            op=OP.subtract)
                    nc.scalar.dma_start(hq_st[half][g_, :, :, :], hq8[:])

                # ========== PHASE A + B under the x-plane pool ==========
                with (
                    tc.tile_pool(name="px", bufs=1) as px,
                    # gate/up matmul PSUM pool spans phases A+B so its banks
                    # are reserved up front: the first phase-B matmul must
                    # not wait for phase-A's transpose-psum pool to close
                    tc.tile_pool(name="pbps", bufs=2, space="PSUM") as pbps,
                ):
                    # x planes split per token-half: half-0 matmuls depend
                    # only on token blocks 0-3's quant, not all of x
                    XH = [px.tile([128, n_hb, half_T], FP8, tag=f"XH{hf}",
                                  name=f"XH{hf}") for hf in range(2)]
                    XL = [px.tile([128, n_hb, half_T], FP8, tag=f"XL{hf}",
                                  name=f"XL{hf}") for hf in range(2)]
                    dg_bt = px.tile([128, Tc], F32, tag="dg_bt")
                    du_bt = px.tile([128, Tc], F32, tag="du_bt")

                    # ---------------- PHASE A ----------------
                    with (
                        tc.tile_pool(name="pa", bufs=2) as pa,
                        tc.tile_pool(name="pa1", bufs=1) as pa1,
                        tc.tile_pool(name="paps", bufs=2,
                                     space="PSUM") as paps,
                    ):
                        # ---- |w| sums for wg+wu+wd, ONE AllReduce ----
                        # Merging wd into the same AR costs ~50us of extra
                        # prepass DMA before AR1 but removes the second AR
                        # and every downstream wait on its result.
                        acc = pa1.tile([128, 4], F32, tag="acc")
                        nc.vector.memset(acc[:], 0.0)
                        srcs = [(0, wg_s[it * 128:(it + 1) * 128, :],
                                 f"g{it}") for it in range(n_ishl)]
                        srcs += [(1, wu_s[it * 128:(it + 1) * 128, :],
                                  f"u{it}") for it in range(n_ishl)]
                        srcs += [(2, wd_s[hb * 128:(hb + 1) * 128, :],
                                  f"d{hb}") for hb in range(n_hb)]
                        for si, (j, src, nm) in enumerate(srcs):
                            wt = pa.tile([128, H], F32, tag="af32a",
                                         name=f"ws{nm}")
                            nc.sync.dma_start(wt[:, :src.shape[1]], src)
                            r = pa.tile([128, 1], F32, tag="rsm",
                                        name=f"wr{nm}")
                            if si % 2 == 0:
                                nc.vector.tensor_reduce(
                                    r[:], wt[:, :src.shape[1]], axis=AX,
                                    op=OP.add, apply_absolute_value=True)
                            else:
                                # Act engine: |x| with free-axis accum sum
                                ab = pa.tile([128, H], F32, tag="af32b",
                                             name=f"wab{nm}")
                                nc.scalar.activation(
                                    ab[:, :src.shape[1]],
                                    wt[:, :src.shape[1]], ACT.Abs,
                                    accum_out=r[:])
                            eng = (nc.vector, nc.gpsimd,
                                   nc.gpsimd)[j]
                            eng.tensor_tensor(acc[:, j:j + 1],
                                              acc[:, j:j + 1], r[:],
                                              op=OP.add)
                        # partition reduce via DMA round-trip (no PE, no PSUM)
                        nc.gpsimd.dma_start(gusum_dr[:, :], acc[:, 0:3])
                        accr2 = pa1.tile([3, 128], F32, tag="accr2")
                        nc.gpsimd.dma_start(
                            accr2[:], gusum_dr.rearrange("p j -> j p"))
                        asum = pa1.tile([3, 1], F32, tag="asum")
                        nc.vector.tensor_reduce(asum[:], accr2[:],
                                                axis=AX, op=OP.add)
                        nc.gpsimd.dma_start(ar_in[0, 0:3], asum[:3])
                        nc.gpsimd.collective_compute(
                            "AllReduce", OP.add, replica_groups=rg,
                            ins=[ar_in[:]], outs=[ar_out[:]])
                        ars = pa1.tile([1, 3], F32, tag="ars")
                        nc.gpsimd.dma_start(ars[:], ar_out[:])
                        sc1 = pa1.tile([1, 3], F32, tag="sc1")
                        nc.vector.tensor_scalar(sc1[:], ars[:], 1.0 / count,
                                                EPS, op0=OP.mult, op1=OP.max)
                        rc1 = pa1.tile([1, 3], F32, tag="rc1")
                        nc.vector.reciprocal(rc1[:], sc1[:])
                        nc.gpsimd.partition_broadcast(scGU[:], sc1[0:1, 0:2])
                        nc.gpsimd.partition_broadcast(rcGU[:], rc1[0:1, 0:2])
                        nc.gpsimd.partition_broadcast(scD[:], sc1[0:1, 2:3])
                        nc.gpsimd.partition_broadcast(rcD[:], rc1[0:1, 2:3])

                        def quant_w_tile(pool, src_ap, rc_ap, nm, width):
                            # ternary weights in BF16 (exact); the PSUM->SBUF
                            # copy after the PE transpose casts to fp8.
                            wt = pool.tile([128, width], F32, tag="af32a",
                                           name=f"wq{nm}")
                            nc.sync.dma_start(wt[:], src_ap)
                            q0 = pool.tile([128, width], F32, tag="af32b",
                                           name=f"q0{nm}")
                            nc.scalar.activation(q0[:], wt[:], ACT.Copy,
                                                 bias=MAGIC, scale=rc_ap)
                            nc.gpsimd.tensor_scalar(q0[:], q0[:], MAGIC, 1.0,
                                                    op0=OP.subtract,
                                                    op1=OP.min)
                            qq = pool.tile([128, width], BF16, tag="aq8",
                                           name=f"qq{nm}")
                            nc.vector.tensor_scalar_max(qq[:], q0[:], -1.0)
                            return qq

                        def emit_wgu_block(k, l):
                            for j, w in ((0, wg_s), (1, wu_s)):
                                qq = quant_w_tile(
                                    pa, w[l * 128:(l + 1) * 128, :],
                                    rcGU[:, j:j + 1], f"gu{j}_{l}", H)
                                ws = pa.tile([128, n_hb, 128], FP8,
                                             tag="asm", name=f"wgus{j}_{l}")
                                # 8 bf16 transposes packed per PSUM bank,
                                # drained with one batched copy each
                                for hg in range(n_hb // 8):
                                    tp = paps.tile([128, 8, 128], BF16,
                                                   tag="tpw",
                                                   name=f"gtp{j}_{l}_{hg}")
                                    for q in range(8):
                                        hb = hg * 8 + q
                                        nc.tensor.transpose(
                                            tp[:, q, :],
                                            qq[:, hb * 128:(hb + 1) * 128],
                                            idb[:])
                                    cp_engines[hg % 2](
                                        ws[:, hg * 8:(hg + 1) * 8, :], tp[:])
                                nc.scalar.dma_start(
                                    wgu8_c[k][:, l - los[k], j, :, :],
                                    ws[:])

                        def emit_x_block(tt):
                            xt = pa.tile([128, H], F32, tag="af32a",
                                         name=f"xt{tt}")
                            nc.sync.dma_start(
                                xt[:], x_s[tt * 128:(tt + 1) * 128, :])
                            amax = pa.tile([128, 1], F32, tag="rsm",
                                           name=f"am{tt}")
                            nc.vector.tensor_reduce(
                                amax[:], xt[:], axis=AX, op=OP.max,
                                apply_absolute_value=True)
                            amc = pa.tile([128, 1], F32, tag="amc",
                                          name=f"amc{tt}")
                            nc.vector.tensor_scalar(amc[:], amax[:], EPS,
                                                    0.0, op0=OP.max,
                                                    op1=OP.add)
                            deq = pa.tile([128, 1], F32, tag="deq",
                                          name=f"dq{tt}")
                            nc.gpsimd.tensor_scalar_mul(deq[:], amc[:],
                                                        1.0 / 127.0)
                            nc.gpsimd.dma_start(
                                deq_row[0, tt * 128:(tt + 1) * 128], deq[:])
                            rec = pa.tile([128, 1], F32, tag="rec",
                                          name=f"rc{tt}")
                            nc.vector.reciprocal(rec[:], amc[:])
                            qs = pa.tile([128, 1], F32, tag="qs",
                                         name=f"qsc{tt}")
                            nc.vector.tensor_scalar_mul(qs[:], rec[:], 127.0)
                            # |x·qs| <= 127 by construction (qs=127/absmax),
                            # so no clamp is needed before the hi/lo split
                            xq = pa.tile([128, H], F32, tag="af32b",
                                         name=f"xq{tt}")
                            nc.scalar.activation(xq[:], xt[:], ACT.Copy,
                                                 bias=MAGIC, scale=qs[:])
                            nc.vector.tensor_scalar(xq[:], xq[:], MAGIC,
                                                    0.0, op0=OP.subtract,
                                                    op1=OP.add)
                            xh32 = pa.tile([128, H], F32, tag="af32a",
                                           name=f"xh32_{tt}")
                            nc.vector.tensor_scalar(xh32[:], xq[:], MAGIC16,
                                                    MAGIC16, op0=OP.add,
                                                    op1=OP.subtract)
                            nc.gpsimd.tensor_tensor(xq[:], xq[:], xh32[:],
                                                    op=OP.subtract)
                            hf, tl = tt // (n_tt // 2), tt % (n_tt // 2)
                            for pl, src in ((0, xh32), (1, xq)):
                                dst = XH[hf] if pl == 0 else XL[hf]
                                # 4 f32 transposes packed per PSUM bank
                                for hg in range(n_hb // 4):
                                    tp = paps.tile([128, 4, 128], F32,
                                                   tag="tpx",
                                                   name=f"xtp{tt}_{pl}_{hg}")
                                    for q in range(4):
                                        hb = hg * 4 + q
                                        nc.tensor.transpose(
                                            tp[:, q, :],
                                            src[:, hb * 128:(hb + 1) * 128],
                                            idf[:])
                                    cp_engines[(hg + pl) % 2](
                                        dst[:, hg * 4:(hg + 1) * 4,
                                            tl * 128:(tl + 1) * 128],
                                        tp[:])

                        # DMA-queue order is the schedule: wgu sums (gate
                        # AR1), then chunk-0 quant re-reads (gate AG0), then
                        # x, then wd sums, then chunks 1-2.  The wd
                        # AllReduce's input side launches in phase A (its
                        # collective slots between AG0 and AG1); its result
                        # readback is deferred to phase B.
                        for tt in range(n_tt // 2):
                            emit_x_block(tt)
                        for k in range(len(chunks)):
                            for l in range(los[k], los[k] + chunks[k]):
                                emit_wgu_block(k, l)
                            nc.gpsimd.collective_compute(
                                "AllGather", OP.bypass, replica_groups=rg,
                                ins=[wgu8_c[k][:]], outs=[wgu8_all[k][:]])
                            if k == 0:
                                for tt in range(n_tt // 2, n_tt):
                                    emit_x_block(tt)

                        # deq scale broadcast rows (local tokens only)
                        dqr = pa1.tile([1, Tc], F32, tag="dqr")
                        nc.gpsimd.dma_start(dqr[:], deq_row[:])
                        dq_bt = pa1.tile([128, Tc], F32, tag="dq_bt")
                        nc.gpsimd.partition_broadcast(dq_bt[:], dqr[:])
                        nc.vector.tensor_scalar_mul(dg_bt[:], dq_bt[:],
                                                    scGU[:, 0:1])
                        nc.gpsimd.tensor_scalar_mul(du_bt[:], dq_bt[:],
                                                    scGU[:, 1:2])


                    # ---------------- PHASE B + wd quant + h quant ------
                    with (
                        tc.tile_pool(name="pbw", bufs=3) as pbw,
                        tc.tile_pool(name="pbe", bufs=2) as pbe,
                        tc.tile_pool(name="pbm", bufs=2) as pbm,
                        tc.tile_pool(name="pbpt", bufs=2,
                                     space="PSUM") as pbpt,
                    ):
                        maxacc = [None, None]

                        def load_wgu(ent, sfx=""):
                            nm = ent_name(ent) + sfx
                            wgu_t = pbw.tile([128, 2, n_hb, 128], FP8,
                                             tag="wgu_t", name=f"wgu{nm}")
                            if ent[0] == "L":
                                _, k, l = ent
                                nc.sync.dma_start(
                                    wgu_t[:],
                                    wgu8_c[k][:, l - los[k], :, :, :])
                            else:
                                _, k, r, l = ent
                                c_dyn = (pid + r) & (N_CORES - 1)
                                nc.sync.dma_start(
                                    wgu_t[:],
                                    wgu8_all[k][c_dyn, :, l - los[k],
                                                :, :, :])
                            return wgu_t

                        def emit_gu_half(half, ent, wgu_t, first):
                            nm = f"{ent_name(ent)}_{half}"
                            sl = slice(half * half_T, (half + 1) * half_T)
                            ps_g = pbps.tile([128, half_T], F32, tag="ps_g",
                                             name=f"psg{nm}")
                            ps_u = pbps.tile([128, half_T], F32, tag="ps_u",
                                             name=f"psu{nm}")
                            for j, ps in ((0, ps_g), (1, ps_u)):
                                for pl, xp in ((0, XH[half]), (1, XL[half])):
                                    for b in range(n_hb // 2):
                                        nc.tensor.matmul(
                                            ps[:],
                                            wgu_t[:, j, 2 * b:2 * b + 2, :],
                                            xp[:, 2 * b:2 * b + 2, :],
                                            start=(pl == 0 and b == 0),
                                            stop=(pl == 1 and
                                                  b == n_hb // 2 - 1),
                                            perf_mode=DR)
                            g = pbe.tile([128, half_T], F32, tag="g",
                                         name=f"g{nm}")
                            nc.vector.tensor_tensor(g[:], ps_g[:],
                                                    dg_bt[:, sl],
                                                    op=OP.mult)
                            sg = pbe.tile([128, half_T], F32, tag="sg",
                                          name=f"sg{nm}")
                            nc.scalar.activation(sg[:], g[:], ACT.Silu)
                            u = pbe.tile([128, half_T], F32, tag="u",
                                         name=f"u{nm}")
                            nc.vector.tensor_tensor(u[:], ps_u[:],
                                                    du_bt[:, sl],
                                                    op=OP.mult)
                            h = pbe.tile([128, half_T], F32, tag="h",
                                         name=f"h{nm}")
                            nc.gpsimd.tensor_tensor(h[:], sg[:], u[:],
                                                    op=OP.mult)
                            nc.scalar.dma_start(
                                h_st[half][ent_slot(ent), :, :], h[:])
                            if first:
                                nc.scalar.activation(
                                    maxacc[half][:], h[:], ACT.Abs)
                            else:
                                ha = pbe.tile([128, half_T], F32, tag="ha",
                                              name=f"ha{nm}")
                                nc.scalar.activation(ha[:], h[:], ACT.Abs)
                                nc.vector.tensor_tensor(
                                    maxacc[half][:], maxacc[half][:], ha[:],
                                    op=OP.max)

                        GW = min(4, n_hb)  # hb per wd store (>=512B runs)

                        def emit_wd_group(gw):
                            ws = pwd.tile([128, n_ishl, GW * 128], FP8,
                                          tag="asmd", name=f"wds{gw}")
                            for q in range(GW):
                                hb = gw * GW + q
                                qq = quant_w_tile(
                                    pwd, wd_s[hb * 128:(hb + 1) * 128, :],
                                    rcD[:, 0:1], f"d_{hb}", Ish)
                                for li, lg in enumerate(
                                        range(0, n_ishl, 8)):
                                    sz = min(8, n_ishl - lg)
                                    tp = pbpt.tile([128, 8, 128], BF16,
                                                   tag="tp8d",
                                                   name=f"dtp{hb}_{lg}")
                                    for i in range(sz):
                                        l = lg + i
                                        nc.tensor.transpose(
                                            tp[:, i, :],
                                            qq[:, l * 128:(l + 1) * 128],
                                            idb[:])
                                    cp_engines[li % 2](
                                        ws[:, lg:lg + sz,
                                           q * 128:(q + 1) * 128],
                                        tp[:, :sz, :])
                            nc.scalar.dma_start(
                                wd8_s[:, :, gw * GW * 128:
                                      (gw + 1) * GW * 128], ws[:])

                        maxacc[0] = pbm.tile([128, half_T], F32,
                                             tag="maxacc", name="mx0")
                        maxacc[1] = pbm.tile([128, half_T], F32,
                                             tag="maxacc", name="mx1")

                        # split order: all-but-last-chunk remotes do both
                        # halves per weight load; the last chunk's remotes
                        # run half 0 first (habs0 early), then half 1 with
                        # the half-0 h quant interleaved underneath.
                        lastk = len(chunks) - 1
                        head = [e for e in order
                                if not (e[0] == "R" and e[1] == lastk)]
                        tailc = [e for e in order
                                 if e[0] == "R" and e[1] == lastk]
                        wd_left = list(range(n_hb // GW))
                        agwd_done = False
                        with tc.tile_pool(name="pwd", bufs=2) as pwd:
                            # wd quant rides after the local window so its
                            # vector-engine work doesn't slow the x quant
                            # that gates the local matmuls
                            for i, ent in enumerate(head):
                                wgu_t = load_wgu(ent)
                                emit_gu_half(0, ent, wgu_t, i == 0)
                                emit_gu_half(1, ent, wgu_t, i == 0)
                                if i >= 22 and i % 2 == 0 and wd_left:
                                    emit_wd_group(wd_left.pop(0))
                                if not wd_left and not agwd_done:
                                    agwd_done = True
                                    nc.gpsimd.collective_compute(
                                        "AllGather", OP.bypass,
                                        replica_groups=rg,
                                        ins=[wd8_s[:]], outs=[wd8_all[:]])
                            # last chunk: half 0 first, then half 1 (weights
                            # re-loaded; the pool can't hold 21 tiles) with
                            # the half-0 h-quant interleaved under it.
                            for ent in tailc:
                                wgu_t = load_wgu(ent)
                                emit_gu_half(0, ent, wgu_t, False)
                                if wd_left:
                                    emit_wd_group(wd_left.pop(0))
                            while wd_left:
                                emit_wd_group(wd_left.pop(0))
                            if not agwd_done:
                                nc.gpsimd.collective_compute(
                                    "AllGather", OP.bypass, replica_groups=rg,
                                    ins=[wd8_s[:]], outs=[wd8_all[:]])
                            emit_habs(0, maxacc[0], pbe, pbpt)
                            q0 = list(range(n_ib))
                            # interleave only the first ~2/3 of the half-0
                            # h quant under the tail's half-1 matmuls; the
                            # rest moves into phase C where it overlaps the
                            # first down-proj matmuls (the hi plane loads in
                            # ih-pieces as its producer blocks quantize)
                            for ent in tailc:
                                wgu_t = load_wgu(ent, "b")
                                emit_gu_half(1, ent, wgu_t, False)
                                for _ in range(2):
                                    if q0:
                                        emit_hq_block(0, q0.pop(0))
                        while len(q0) > n_ib // 2 - 1:
                            emit_hq_block(0, q0.pop(0))
                        emit_habs(1, maxacc[1], pbe, pbpt)

                # ================= PHASE C: down =================
                # wd is streamed in half-I tiles with >=512B DMA runs; the
                # PSUM groups of all 4 token blocks of an H-chunk stay open
                # across both I-halves.
                HC2 = min(512, H)
                n_hc2 = H // HC2
                nih = 2 if n_ishl * (N_CORES // 2) * 2 == n_ib and \
                    N_CORES % 2 == 0 else 1
                ihb = n_ib // nih
                n_tb = half_T // 128
                with (
                    tc.tile_pool(name="pchh", bufs=1) as pchh,
                    tc.tile_pool(name="pchl", bufs=1) as pchl,
                    tc.tile_pool(name="pcw", bufs=3) as pcw,
                    tc.tile_pool(name="pcd", bufs=2) as pcd,
                    tc.tile_pool(name="pcps", bufs=2, space="PSUM") as pcps,
                ):
                    qrem = list(range(n_ib))
                    drain = (n_ib + n_hc2 - 1) // n_hc2
                    # plane-outer matmul order: each tg's matmuls start
                    # after only the hi-plane load; the lo plane loads
                    # under the hi-plane matmuls.  (PSUM accumulation is
                    # exact integer, so reordering is bit-identical.)
                    for tg in range(2):
                        hq_hi = pchh.tile([128, n_ib, half_T], FP8,
                                          tag="hq_hi", name=f"hqh{tg}")
                        for ih in range(nih):
                            blo = ih * ihb
                            if tg == 0 and ih == 1:
                                # finish the half-0 h quant (blocks of the
                                # second ih piece) before loading that piece
                                while q0:
                                    emit_hq_block(0, q0.pop(0))
                            nc.sync.dma_start(
                                hq_hi[:, blo:blo + ihb, :],
                                hq_v[tg][0, :, blo:blo + ihb, :])
                        hq_lo = pchl.tile([128, n_ib, half_T], FP8,
                                          tag="hq_lo", name=f"hql{tg}")
                        for ih in range(nih):
                            blo = ih * ihb
                            nc.sync.dma_start(
                                hq_lo[:, blo:blo + ihb, :],
                                hq_v[tg][1, :, blo:blo + ihb, :])
                        hqp = (hq_hi, hq_lo)
                        for hc in range(n_hc2):
                            wdt = []
                            for ih in range(nih):
                                w = pcw.tile([128, ihb, HC2], FP8,
                                             tag="wd_t",
                                             name=f"wd{tg}_{hc}_{ih}")
                                c0 = ih * (N_CORES // nih)
                                for c in range(c0, c0 + N_CORES // nih):
                                    lo = (c - c0) * n_ishl
                                    nc.sync.dma_start(
                                        w[:, lo:lo + n_ishl, :],
                                        wd8_all[c, :, :,
                                                hc * HC2:(hc + 1) * HC2])
                                wdt.append(w)
                            pss = [pcps.tile([128, HC2], F32,
                                             tag=f"psy{tb}",
                                             name=f"psy{tg}_{hc}_{tb}")
                                   for tb in range(n_tb)]
                            for pl in range(2):
                                for ih in range(nih):
                                    for tb in range(n_tb):
                                        tsl = slice(tb * 128,
                                                    (tb + 1) * 128)
                                        for p in range(ihb // 2):
                                            gp = ih * (ihb // 2) + p
                                            nc.tensor.matmul(
                                                pss[tb][:],
                                                hqp[pl][:, 2 * gp:2 * gp + 2,
                                                        tsl],
                                                wdt[ih][:, 2 * p:2 * p + 2,
                                                        :],
                                                start=(pl == 0 and ih == 0
                                                       and p == 0),
                                                stop=(pl == 1 and
                                                      ih == nih - 1 and
                                                      p == ihb // 2 - 1),
                                                perf_mode=DR)
                            for tb in range(n_tb):
                                yv = pcd.tile([128, HC2], F32, tag="yv",
                                              name=f"yv{tg}_{hc}_{tb}")
                                tcol = tg * n_tb + tb
                                nc.vector.tensor_scalar_mul(
                                    yv[:], pss[tb][:],
                                    dv8[:, tcol:tcol + 1])
                                nc.sync.dma_start(
                                    y[tg * half_T + tb * 128:
                                      tg * half_T + (tb + 1) * 128,
                                      hc * HC2:(hc + 1) * HC2],
                                    yv[:])
                            if tg == 0:
                                for _ in range(drain):
                                    if qrem:
                                        emit_hq_block(1, qrem.pop(0))
                        while tg == 0 and qrem:
                            emit_hq_block(1, qrem.pop(0))

    nc.compile()
    return nc


_CACHE = {}


def _get_program():
    if "full" not in _CACHE:
        _CACHE["full"] = build_program(**FULL_CFG)
    return _CACHE["full"]


def kernel(x, w_gate, w_up, w_down):
    B, S, H = x.shape
    I = w_gate.shape[0]
    T = B * S
    Tc = T // N_CORES
    Ish = FULL_CFG["Ish"]
    Ipad = Ish * N_CORES

    xf = np.ascontiguousarray(np.asarray(x, np.float32).reshape(T, H))
    wg_pad = np.zeros((Ipad, H), np.float32)
    wg_pad[:I] = np.asarray(w_gate, np.float32)
    wu_pad = np.zeros((Ipad, H), np.float32)
    wu_pad[:I] = np.asarray(w_up, np.float32)
    wd_pad = np.zeros((H, Ipad), np.float32)
    wd_pad[:, :I] = np.asarray(w_down, np.float32)

    in_maps = []
    for c in range(N_CORES):
        in_maps.append({
            "x_s": np.ascontiguousarray(xf[c * Tc:(c + 1) * Tc]),
            "wg_s": np.ascontiguousarray(wg_pad[c * Ish:(c + 1) * Ish]),
            "wu_s": np.ascontiguousarray(wu_pad[c * Ish:(c + 1) * Ish]),
            "wd_s": np.ascontiguousarray(wd_pad[:, c * Ish:(c + 1) * Ish]),
        })

    nc = _get_program()
    res = run_bass_kernel_spmd(nc, in_maps, core_ids=list(range(N_CORES)))
    out = np.concatenate([res.results[c]["y"] for c in range(N_CORES)], axis=0)
    return out.reshape(B, S, H).astype(np.float32)
